# revision 14
# baseline (speedup 1.0000x reference)
"""Trainium2 Bass kernel for nn_Context_ComplEx_v3 (gnn_message_passing).

Sharding (per spec hint):
  - entity table for the final score matmul: column-sharded over n_ent across
    the 8 cores (each core owns a contiguous 50k-entity slab, stored
    transposed [2r, 50000] so the PE contraction streams contiguously),
  - batch data-parallel for the query/attention computation (64 rows/core),
  - small W/Wo/Uo params replicated,
  - one AllGather of the [B, 2r] query vectors.

Host does only sharding-style work: row gathers from the embedding tables,
slicing/concats/transposes for layout, and parameter repacking. All math
(matmuls, attention, softmax, gating, score matmul, norms) runs on device.
"""
import os
import sys
import types
from contextlib import ExitStack
from functools import lru_cache

sys.path.insert(0, "/opt/trn_rl_repo")

import numpy as np


# --------------------------------------------------------------------------
# Environment shims: NTFF-profile hook (image's antenv lacks axon_hooks).
# --------------------------------------------------------------------------
def _install_ntff_hook():
    if "antenv.axon_hooks" in sys.modules:
        return
    mod = types.ModuleType("antenv.axon_hooks")
    _state = {"hook": None}
    mod.set_axon_ntff_profile_hook = lambda h: _state.__setitem__("hook", h)
    mod.get_axon_ntff_profile_hook = lambda: _state["hook"]
    sys.modules["antenv.axon_hooks"] = mod
    import antenv

    antenv.axon_hooks = mod
    try:
        from trn_agent_boot.trn_boot import _ntff_profile_via_ctypes

        mod.set_axon_ntff_profile_hook(_ntff_profile_via_ctypes("/opt/axon/libaxon_pjrt.so"))
    except Exception:
        pass


_install_ntff_hook()

import concourse.bass as bass
import concourse.tile as tile
from concourse import mybir
from concourse.bass_utils import run_bass_kernel_spmd
from concourse.masks import make_identity
from concourse import bacc



# --------------------------------------------------------------------------
# Problem constants (hardcoded per the task contract).
# --------------------------------------------------------------------------
N_ENT = 400000
R = 128
B = 512
M = 50
NCORES = 8
ESH = N_ENT // NCORES  # 50000 entities per core
BL = B // NCORES  # 64 batch rows per core
K2 = 2 * R  # 256
E_TILE = 512  # matmul N tile (one PSUM bank of fp32)
E_CHUNK = 2048  # entities per DMA chunk / out-buffer flush

F32 = mybir.dt.float32
F32R = mybir.dt.float32r
F16 = mybir.dt.float16
Q_SCALE = float(2 ** 20)
ENT_SCALE = float(2 ** 10)
DESCALE = float(2 ** -30)


def _r(ap):
    """View an fp32 AP as float32r so the PE runs single-pass FP22 matmul."""
    return ap


@lru_cache(maxsize=4)
def _build_bass(esh=ESH, phases=None, kskip=None):
    if phases is None:
        phases = os.environ.get("KPHASES", "AB")
    if kskip is None:
        kskip = os.environ.get("KSKIP", "")
    skip = set(kskip.split(","))
    nc = bacc.Bacc("TRN2", target_bir_lowering=False, debug=False, num_devices=NCORES)

    # ---- per-core I/O ----
    entT = nc.dram_tensor("entT", [K2, esh], F16, kind="ExternalInput").ap()
    trp = nc.dram_tensor("trp", [BL, 4 * R], F32, kind="ExternalInput").ap()
    trpT = nc.dram_tensor("trpT", [4 * R, BL], F32, kind="ExternalInput").ap()
    wzw = nc.dram_tensor("wzw", [4 * R, K2], F32, kind="ExternalInput").ap()
    wzb = nc.dram_tensor("wzb", [BL, K2], F32, kind="ExternalInput").ap()
    nbt = nc.dram_tensor("nbt", [BL, M * K2], F32, kind="ExternalInput").ap()
    rhst = nc.dram_tensor("rhst", [BL, K2], F32, kind="ExternalInput").ap()
    uo0 = nc.dram_tensor("uo0", [BL, R], F32, kind="ExternalInput").ap()
    nuo1 = nc.dram_tensor("nuo1", [BL, R], F32, kind="ExternalInput").ap()
    wo0 = nc.dram_tensor("wo0", [BL, R], F32, kind="ExternalInput").ap()
    bgr = nc.dram_tensor("bgr", [BL, 1], F32, kind="ExternalInput").ap()

    scores = nc.dram_tensor("scores", [B, esh], F32, kind="ExternalOutput").ap()
    f_lhs = nc.dram_tensor("f_lhs", [BL, R], F32, kind="ExternalOutput").ap()
    f_rel = nc.dram_tensor("f_rel", [BL, R], F32, kind="ExternalOutput").ap()
    f_rhs = nc.dram_tensor("f_rhs", [BL, R], F32, kind="ExternalOutput").ap()
    f_gec = nc.dram_tensor("f_gec", [BL, R], F32, kind="ExternalOutput").ap()

    entT3 = entT.rearrange("(o p) e -> p o e", p=128)  # [128, 2, esh]

    with tile.TileContext(nc) as tc, ExitStack() as ctx:
        const = ctx.enter_context(tc.tile_pool(name="const", bufs=1))
        rhs_pool = ctx.enter_context(tc.tile_pool(name="rhs_pool", bufs=3))
        out_pool = ctx.enter_context(tc.tile_pool(name="out_pool", bufs=2))

        ident = const.tile([128, 128], F32)
        make_identity(nc, ident)
        qT0 = const.tile([128, B], F16)
        qT1 = const.tile([128, B], F16)
        qT = [qT0, qT1]

        # =============== Phase A: queries (batch-parallel) ===============
        if "A" not in phases:
            # bisect mode: fill qT with junk via a broadcastish DMA
            for t in (qT0, qT1):
                for j in range(8):
                    nc.sync.dma_start(
                        t[:, j * 64 : (j + 1) * 64],
                        trpT.rearrange("(o p) m -> p o m", p=128)[:, 0],
                    )
        if "A" in phases:
          with tc.tile_pool(name="attn", bufs=1) as work:
            trp_sb = work.tile([BL, 4 * R], F32)
            nc.sync.dma_start(trp_sb[:], trp[:, :])
            trpT_sb = work.tile([128, 4, BL], F32)
            nc.sync.dma_start(trpT_sb[:], trpT.rearrange("(o p) m -> p o m", p=128))
            wzw_sb = work.tile([128, 4, K2], F32)
            nc.sync.dma_start(wzw_sb[:], wzw.rearrange("(o p) n -> p o n", p=128))
            wzb_sb = work.tile([BL, K2], F32)
            nc.sync.dma_start(wzb_sb[:], wzb[:, :])
            nb_sb = work.tile([BL, M, K2], F32)
            nc.sync.dma_start(nb_sb[:], nbt.rearrange("b (m k) -> b m k", k=K2))
            rhst_sb = work.tile([BL, K2], F32)
            nc.sync.dma_start(rhst_sb[:], rhst[:, :])
            uo0_sb = work.tile([BL, R], F32)
            nc.sync.dma_start(uo0_sb[:], uo0[:, :])
            nuo1_sb = work.tile([BL, R], F32)
            nc.sync.dma_start(nuo1_sb[:], nuo1[:, :])
            wo0_sb = work.tile([BL, R], F32)
            nc.sync.dma_start(wo0_sb[:], wo0[:, :])
            bg_sb = work.tile([BL, 1], F32)
            nc.sync.dma_start(bg_sb[:], bgr[:, :])

            # wz = [w_r | -w_i] = trp_cat @ Wz + bias_z
            wz_sb = work.tile([BL, K2], F32)
            with tc.tile_pool(name="psA", bufs=1, space="PSUM") as psA:
                wz_ps = psA.tile([BL, K2], F32)
                for kt in range(4):
                    nc.tensor.matmul(
                        wz_ps[:],
                        trpT_sb[:, kt],
                        wzw_sb[:, kt],
                        start=(kt == 0),
                        stop=(kt == 3),
                    )
                nc.vector.tensor_add(wz_sb[:], wz_ps[:], wzb_sb[:])

            # w_nb[b, m] = sum_k wz[b, k] * nb[b, m, k]
            wnb = work.tile([BL, M], F32)
            HM = M // 2  # 25
            tmp_h = work.tile([BL, HM, K2], F32)
            for h in range(2):
                nc.vector.tensor_tensor(
                    tmp_h[:],
                    nb_sb[:, h * HM : (h + 1) * HM],
                    wz_sb[:, None, :].to_broadcast([BL, HM, K2]),
                    mybir.AluOpType.mult,
                )
                nc.vector.tensor_reduce(
                    wnb[:, h * HM : (h + 1) * HM],
                    tmp_h[:],
                    axis=mybir.AxisListType.X,
                    op=mybir.AluOpType.add,
                )

            # masked softmax weights p (unnormalized) + 1/denom
            mask_inv = work.tile([BL, M], F32)
            nc.vector.tensor_single_scalar(
                mask_inv[:], wnb[:], 0.0, op=mybir.AluOpType.not_equal
            )
            negmax = work.tile([BL, 1], F32)
            nc.vector.tensor_reduce(
                negmax[:], wnb[:], axis=mybir.AxisListType.X, op=mybir.AluOpType.max, negate=True
            )
            e_t = work.tile([BL, M], F32)
            nc.scalar.activation(
                e_t[:], wnb[:], mybir.ActivationFunctionType.Exp, bias=negmax[:]
            )
            p_t = work.tile([BL, M], F32)
            denom = work.tile([BL, 1], F32)
            nc.vector.scalar_tensor_tensor(
                out=p_t[:],
                in0=e_t[:],
                scalar=1.0,
                in1=mask_inv[:],
                op0=mybir.AluOpType.mult,
                op1=mybir.AluOpType.mult,
                accum_out=denom[:],
            )
            recip = work.tile([BL, 1], F32)
            nc.vector.reciprocal(recip[:], denom[:])

            # ec = (sum_m p_m * nb_m) / denom   (both halves at once: [ec_r|ec_i])
            acc = work.tile([BL, K2], F32)
            nc.vector.tensor_scalar_mul(acc[:], nb_sb[:, 0], p_t[:, 0:1])
            for m in range(1, M):
                nc.vector.scalar_tensor_tensor(
                    out=acc[:],
                    in0=nb_sb[:, m],
                    scalar=p_t[:, m : m + 1],
                    in1=acc[:],
                    op0=mybir.AluOpType.mult,
                    op1=mybir.AluOpType.add,
                )
            ec = work.tile([BL, K2], F32)
            nc.vector.tensor_scalar_mul(ec[:], acc[:], recip[:])
            ec_r, ec_i = ec[:, :R], ec[:, R:]

            # gating scalar g
            lr, rr = trp_sb[:, 0:R], trp_sb[:, R : 2 * R]
            li, ri = trp_sb[:, 2 * R : 3 * R], trp_sb[:, 3 * R :]
            srrr = work.tile([BL, R], F32)
            nc.vector.tensor_mul(srrr[:], lr, rr)
            siri = work.tile([BL, R], F32)
            nc.vector.tensor_mul(siri[:], li, ri)
            sirr = work.tile([BL, R], F32)
            nc.vector.tensor_mul(sirr[:], li, rr)
            srri = work.tile([BL, R], F32)
            nc.vector.tensor_mul(srri[:], lr, ri)
            a1 = work.tile([BL, R], F32)
            nc.vector.tensor_sub(a1[:], srrr[:], siri[:])
            a2 = work.tile([BL, R], F32)
            nc.vector.tensor_add(a2[:], sirr[:], srri[:])

            junkR = work.tile([BL, R], F32)
            t1 = work.tile([BL, 1], F32)
            t2 = work.tile([BL, 1], F32)
            t3 = work.tile([BL, 1], F32)
            nc.vector.tensor_mul(junkR[:], a1[:], uo0_sb[:])
            nc.vector.tensor_reduce(t1[:], junkR[:], axis=mybir.AxisListType.X, op=mybir.AluOpType.add)
            nc.vector.tensor_mul(junkR[:], a2[:], nuo1_sb[:])
            nc.vector.tensor_reduce(t2[:], junkR[:], axis=mybir.AxisListType.X, op=mybir.AluOpType.add)
            nc.vector.tensor_mul(junkR[:], ec_r, wo0_sb[:])
            nc.vector.tensor_reduce(t3[:], junkR[:], axis=mybir.AxisListType.X, op=mybir.AluOpType.add)
            gpre = work.tile([BL, 1], F32)
            nc.vector.tensor_add(t1[:], t1[:], t2[:])
            nc.vector.tensor_add(t1[:], t1[:], t3[:])
            nc.vector.tensor_add(gpre[:], t1[:], bg_sb[:])
            g = work.tile([BL, 1], F32)
            nc.scalar.activation(g[:], gpre[:], mybir.ActivationFunctionType.Sigmoid)
            omg = work.tile([BL, 1], F32)
            nc.vector.tensor_scalar(
                omg[:], g[:], -1.0, 1.0, op0=mybir.AluOpType.mult, op1=mybir.AluOpType.add
            )

            gecr = work.tile([BL, R], F32)
            nc.vector.tensor_scalar(
                gecr[:], ec_r, g[:, 0:1], omg[:, 0:1],
                op0=mybir.AluOpType.mult, op1=mybir.AluOpType.add,
            )
            geci = work.tile([BL, R], F32)
            nc.vector.tensor_scalar_mul(geci[:], ec_i, g[:, 0:1])

            # q = [qr | qi]
            q_loc = work.tile([BL, K2], F32)
            tA = work.tile([BL, R], F32)
            tB = work.tile([BL, R], F32)
            nc.vector.tensor_mul(tA[:], a1[:], gecr[:])
            nc.vector.tensor_mul(tB[:], a2[:], geci[:])
            nc.vector.tensor_add(q_loc[:, :R], tA[:], tB[:])
            nc.vector.tensor_mul(tA[:], a2[:], gecr[:])
            nc.vector.tensor_mul(tB[:], a1[:], geci[:])
            nc.vector.tensor_sub(q_loc[:, R:], tA[:], tB[:])

            # f_* norms: sqrt(re^2 + im^2)
            fsb = work.tile([BL, R], F32)
            for re_ap, im_ap, out_dram in (
                (lr, li, f_lhs),
                (rr, ri, f_rel),
                (rhst_sb[:, :R], rhst_sb[:, R:], f_rhs),
                (gecr[:], geci[:], f_gec),
            ):
                nc.vector.tensor_mul(tA[:], re_ap, re_ap)
                nc.vector.tensor_mul(tB[:], im_ap, im_ap)
                nc.vector.tensor_add(fsb[:], tA[:], tB[:])
                fout = work.tile([BL, R], F32, tag="fout")
                nc.scalar.activation(fout[:], fsb[:], mybir.ActivationFunctionType.Sqrt)
                nc.sync.dma_start(out_dram[:, :], fout[:])

            # ---- AllGather q: [64, 256] per core -> [512, 256] everywhere ----
            q_sb = work.tile([128, 4, K2], F32)
            if "ag" not in skip:
                with tc.tile_pool(name="dramq", bufs=1, space="DRAM") as dramq:
                    q_in = dramq.tile([BL, K2], F32)
                    q_all = dramq.tile([B, K2], F32, addr_space="Shared")
                    nc.sync.dma_start(q_in[:], q_loc[:])
                    nc.gpsimd.collective_compute(
                        "AllGather",
                        mybir.AluOpType.bypass,
                        replica_groups=[list(range(NCORES))],
                        ins=[q_in[:].opt()],
                        outs=[q_all[:].opt()],
                    )
                    nc.sync.dma_start(q_sb[:], q_all.rearrange("(o p) k -> p o k", p=128))
            else:
                with tc.tile_pool(name="dramq2", bufs=1, space="DRAM") as dramq2:
                    q_one = dramq2.tile([BL, K2], F32)
                    nc.sync.dma_start(q_one[:], q_loc[:])
                    for o in range(4):
                        nc.sync.dma_start(q_sb[:64, o], q_one[:, :])
                        nc.sync.dma_start(q_sb[64:, o], q_one[:, :])

            # transpose q -> qT tiles [128k, 512b] (2 k-tiles)
            if "tr" not in skip:
                with tc.tile_pool(name="psT", bufs=2, space="PSUM") as psT:
                    for bt in range(4):
                        for kt in range(2):
                            pst = psT.tile([128, 128], F32, tag="pst")
                            nc.tensor.transpose(
                                pst[:], q_sb[:, bt, kt * 128 : (kt + 1) * 128], ident[:]
                            )
                            nc.vector.tensor_scalar_mul(
                                qT[kt][:, bt * 128 : (bt + 1) * 128], pst[:], Q_SCALE
                            )
            else:
                for kt in range(2):
                    for bt in range(4):
                        nc.vector.tensor_scalar_mul(
                            qT[kt][:, bt * 128 : (bt + 1) * 128],
                            q_sb[:, bt, kt * 128 : (kt + 1) * 128],
                            Q_SCALE,
                        )

        # =============== Phase B: scores = q @ entT (streamed) ===============
        psum = ctx.enter_context(tc.tile_pool(name="ps", bufs=2, space="PSUM"))

        chunk_starts = list(range(0, esh, E_CHUNK)) if "B" in phases else []
        for c0 in chunk_starts:
            csz = min(E_CHUNK, esh - c0)
            rhs_sb = rhs_pool.tile([128, 2, E_CHUNK], F16, tag="rhs")
            nc.sync.dma_start(rhs_sb[:, :, :csz], entT3[:, :, c0 : c0 + csz])
            outs = [
                out_pool.tile([128, E_CHUNK], F32, tag=f"o{bt}", name=f"o{bt}")
                for bt in range(4)
            ]
            for off in range(0, csz, E_TILE):
                s = min(E_TILE, csz - off)
                for bt in range(4):
                    ps = psum.tile([128, E_TILE], F32, tag=f"p{bt}")
                    nc.tensor.matmul(
                        ps[:, :s],
                        qT0[:, bt * 128 : (bt + 1) * 128],
                        rhs_sb[:, 0, off : off + s],
                        start=True,
                        stop=False,
                    )
                    nc.tensor.matmul(
                        ps[:, :s],
                        qT1[:, bt * 128 : (bt + 1) * 128],
                        rhs_sb[:, 1, off : off + s],
                        start=False,
                        stop=True,
                    )
                    if bt < 2:
                        nc.vector.tensor_scalar_mul(
                            outs[bt][:, off : off + s], ps[:, :s], DESCALE
                        )
                    else:
                        nc.scalar.mul(outs[bt][:, off : off + s], ps[:, :s], DESCALE)
            for bt in range(4):
                nc.sync.dma_start(
                    scores[bt * 128 : (bt + 1) * 128, c0 : c0 + csz],
                    outs[bt][:, :csz],
                )

    nc.compile()
    return nc


# --------------------------------------------------------------------------
# Host-side: shard, gather, run, unshard.
# --------------------------------------------------------------------------
def kernel(ent_emb, rel_emb, obj_emb, W0, W1, bw0, bw1, Wo0, Uo0, Uo1, b_g, x, nb_idx):
    ent_emb = np.ascontiguousarray(np.asarray(ent_emb, dtype=np.float32))
    rel_emb = np.asarray(rel_emb, dtype=np.float32)
    obj_emb = np.asarray(obj_emb, dtype=np.float32)
    W0 = np.asarray(W0, dtype=np.float32)
    W1 = np.asarray(W1, dtype=np.float32)
    bw0 = np.asarray(bw0, dtype=np.float32)
    bw1 = np.asarray(bw1, dtype=np.float32)
    Wo0 = np.asarray(Wo0, dtype=np.float32)
    Uo0 = np.asarray(Uo0, dtype=np.float32)
    Uo1 = np.asarray(Uo1, dtype=np.float32)
    b_g = np.asarray(b_g, dtype=np.float32)
    x = np.asarray(x)
    nb_idx = np.asarray(nb_idx)

    # row gathers (this is the data-parallel input distribution)
    lhs = ent_emb[x[:, 0]]  # [B, 2r]
    rel = rel_emb[x[:, 1]]
    rhs = ent_emb[x[:, 2]]
    trp = np.concatenate([lhs[:, :R], rel[:, :R], lhs[:, R:], rel[:, R:]], axis=1)  # [B, 4r]
    nb = obj_emb[nb_idx.reshape(-1)].reshape(B, M * K2)  # [B, 50*256]

    # replicated params, repacked
    Wz = np.block([[W0, -W1], [-W1, -W0]]).astype(np.float32)  # [4r, 2r]
    bias_z = np.concatenate([bw0, -bw1], axis=1)  # [1, 2r]
    wzb = np.ascontiguousarray(np.broadcast_to(bias_z, (BL, K2)))
    uo0 = np.ascontiguousarray(np.broadcast_to(Uo0.reshape(1, R), (BL, R)))
    nuo1 = np.ascontiguousarray(np.broadcast_to(-Uo1.reshape(1, R), (BL, R)))
    wo0 = np.ascontiguousarray(np.broadcast_to(Wo0.reshape(1, R), (BL, R)))
    bgr = np.full((BL, 1), float(b_g[0, 0]), dtype=np.float32)

    in_maps = []
    for c in range(NCORES):
        b0, b1 = c * BL, (c + 1) * BL
        e0, e1 = c * ESH, (c + 1) * ESH
        in_maps.append(
            {
                "entT": np.ascontiguousarray((ent_emb[e0:e1].T * ENT_SCALE).astype(np.float16)),
                "trp": np.ascontiguousarray(trp[b0:b1]),
                "trpT": np.ascontiguousarray(trp[b0:b1].T),
                "wzw": Wz,
                "wzb": wzb,
                "nbt": np.ascontiguousarray(nb[b0:b1]),
                "rhst": np.ascontiguousarray(rhs[b0:b1]),
                "uo0": uo0,
                "nuo1": nuo1,
                "wo0": wo0,
                "bgr": bgr,
            }
        )

    nc = _build_bass()
    res = run_bass_kernel_spmd(
        nc,
        in_maps,
        core_ids=list(range(NCORES)),
        trace=bool(int(os.environ.get("KERNEL_TRACE", "0"))),
    )
    kernel.last_results = res

    scores = np.concatenate([res.results[c]["scores"] for c in range(NCORES)], axis=1)
    f_lhs = np.concatenate([res.results[c]["f_lhs"] for c in range(NCORES)], axis=0)
    f_rel = np.concatenate([res.results[c]["f_rel"] for c in range(NCORES)], axis=0)
    f_rhs = np.concatenate([res.results[c]["f_rhs"] for c in range(NCORES)], axis=0)
    f_gec = np.concatenate([res.results[c]["f_gec"] for c in range(NCORES)], axis=0)
    return scores, f_lhs, f_rel, f_rhs, f_gec
